# revision 11
# baseline (speedup 1.0000x reference)
"""DHCNN Trainium2 kernel.

Model (see reference): D=3 stacked HCNN levels over T=320 timesteps (256 past
teacher-forced + 64 forecast), state size NS=512, NY=32 observed features,
B=256 batch. Level 0 is an LSTM-gated cell: s' = d*(tanh(sc)@A0.T) + (1-d)*sc;
levels 1,2: s' = tanh(sc)@Ai.T. Teacher forcing (p=1) replaces the first NY
state features with the observation y_t; err_t = s_in[:NY] - y_t is the output
for past steps, fc_t = s_in[:NY] for forecast steps. Level i+1's input stream
is level i's pre-cell state s_in.

Mapping: data-parallel over batch, 8 cores x Bc=32. On-chip layout is
feature-on-partition: state = (128 partitions, 4 chunks * 32 batch) fp32 tile,
chunk kc covering features [kc*128,(kc+1)*128). The per-step matmul
upd.T = A @ tanh(sc).T runs weights-stationary: 16 bf16 matmuls per level-step
(lhsT = A.T tile (128,128), moving = tanh chunk (128,32)), accumulating over
kc into a (128,128) PSUM tile. The 3 levels run as a wavefront (level l
processes t = g - l at global step g) so their dependency chains overlap
across engines. Level-0 gate blend uses the "w-form": w = (1-d) * s_corr is
computed off the critical chain on GpSimd (teacher rows come from the
precomputed dyt = (1-d[:NY]) * y_t), so the post-matmul tail is just
q = d*ps; nxt = q + w on DVE. All transposes/casts happen on the host.
"""
import sys

sys.path.insert(0, "/opt/trn_rl_repo")
sys.path.insert(0, "/opt/pypackages")

from contextlib import ExitStack

import ml_dtypes
import numpy as np

import concourse.bacc as bacc
import concourse.tile as tile
from concourse import mybir
from concourse.bass_utils import run_bass_kernel_spmd

D, T_PAST, T_FC, NS, NY, B = 3, 256, 64, 512, 32, 256
T = T_PAST + T_FC
NCORES = 8
Bc = B // NCORES  # 32
KC = NS // 128  # 4 feature chunks
AF = mybir.ActivationFunctionType
OP = mybir.AluOpType

_nc_cache = None


def build_nc(variant=2):
    nc = bacc.Bacc("TRN2", target_bir_lowering=False)
    f32, bf16 = mybir.dt.float32, mybir.dt.bfloat16

    at_h = nc.declare_dram_parameter("at_h", [D, NS, NS], bf16, isOutput=False)
    ty_h = nc.declare_dram_parameter("ty_h", [NY, T_PAST, Bc], bf16, isOutput=False)
    yt_h = nc.declare_dram_parameter("yt_h", [NY, T_PAST, Bc], f32, isOutput=False)
    dyt_h = nc.declare_dram_parameter("dyt_h", [NY, T_PAST, Bc], f32, isOutput=False)
    efull_h = nc.declare_dram_parameter("efull_h", [128, KC * Bc], f32, isOutput=False)
    init_h = nc.declare_dram_parameter("init_h", [128, D * KC], f32, isOutput=False)
    eo_h = nc.declare_dram_parameter("eo_h", [D, NY, T, Bc], f32, isOutput=True)

    EB = 32  # err-stage block (timesteps per output DMA)

    with tile.TileContext(nc) as tc, ExitStack() as ctx:
        const = ctx.enter_context(tc.tile_pool(name="const", bufs=1))
        state = ctx.enter_context(tc.tile_pool(name="state", bufs=1))
        psum = ctx.enter_context(tc.tile_pool(name="psum", bufs=1, space="PSUM"))

        # ---- resident constants (level-0 weights first so compute starts early) ----
        at = [[None] * KC for _ in range(D)]
        for kc in range(KC):
            w = const.tile([128, NS], bf16, name=f"at0_{kc}", tag=f"at0_{kc}")
            nc.sync.dma_start(w[:], at_h[0, kc * 128 : (kc + 1) * 128, :])
            at[0][kc] = w
        ty = const.tile([NY, T_PAST * Bc], bf16, name="ty")
        nc.sync.dma_start(ty[:], ty_h[:, :, :])
        yt = const.tile([NY, T_PAST * Bc], f32, name="yt")
        nc.sync.dma_start(yt[:], yt_h[:, :, :])
        dyt = const.tile([NY, T_PAST * Bc], f32, name="dyt")
        nc.sync.dma_start(dyt[:], dyt_h[:, :, :])
        efull = const.tile([128, KC * Bc], f32, name="efull")
        nc.sync.dma_start(efull[:], efull_h[:, :])
        init = const.tile([128, D * KC], f32, name="init")
        nc.sync.dma_start(init[:], init_h[:, :])
        for l in range(1, D):
            for kc in range(KC):
                w = const.tile([128, NS], bf16, name=f"at{l}_{kc}", tag=f"at{l}_{kc}")
                nc.sync.dma_start(w[:], at_h[l, kc * 128 : (kc + 1) * 128, :])
                at[l][kc] = w

        # ---- persistent state ----
        ring1 = [state.tile([128, KC * Bc], f32, name=f"ring1_{s}", tag=f"ring1_{s}") for s in range(3)]
        ring2 = [state.tile([128, KC * Bc], f32, name=f"ring2_{s}", tag=f"ring2_{s}") for s in range(3)]
        zeros = state.tile([128, KC * Bc], f32, name="zeros", tag="zeros")
        nc.vector.memset(zeros[:], 0)
        init_bc = []
        for l in range(D):
            bc = state.tile([128, KC * Bc], f32, name=f"init_bc{l}", tag=f"init_bc{l}")
            for kc in range(KC):
                nc.vector.tensor_scalar_add(
                    bc[:, kc * Bc : (kc + 1) * Bc],
                    zeros[:, kc * Bc : (kc + 1) * Bc],
                    init[:, l * KC + kc : l * KC + kc + 1],
                )
            init_bc.append(bc)
        nc.vector.tensor_copy(ring1[0][:], init_bc[0][:])

        # ---- per-step pools ----
        thp = [ctx.enter_context(tc.tile_pool(name=f"th{l}", bufs=2)) for l in range(D)]
        psp = [ctx.enter_context(tc.tile_pool(name=f"ps{l}", bufs=2, space="PSUM")) for l in range(D)]
        wp = ctx.enter_context(tc.tile_pool(name="wp", bufs=2))
        sin2p = ctx.enter_context(tc.tile_pool(name="sin2p", bufs=2))
        errp = [ctx.enter_context(tc.tile_pool(name=f"err{l}", bufs=2)) for l in range(D)]

        ps_prev = [None, None, None]
        err_tile = [None, None, None]

        # w for level-0 step 0: (1-d) * s_corr(0); teacher rows from dyt[0]
        w_cur = wp.tile([128, KC * Bc], f32, name="w", tag="w")
        nc.gpsimd.tensor_mul(w_cur[:], efull[:], ring1[0][:])
        nc.gpsimd.tensor_copy(w_cur[0:NY, 0:Bc], dyt[:, 0:Bc])

        for g in range(T + D - 1):
            for l in range(D):
                t = g - l
                if not (0 <= t < T):
                    continue
                past = t < T_PAST
                tb = slice(t * Bc, (t + 1) * Bc)

                # --- s_in ---
                if l == 0:
                    s_in = ring1[t % 3]
                elif l == 1:
                    s_in = ring2[t % 3]
                    low = ring1[t % 3]
                    src = init_bc[1] if t == 0 else ps_prev[1]
                    nc.vector.tensor_add(s_in[:], src[:], low[:])
                else:
                    s_in = sin2p.tile([128, KC * Bc], f32, name="sin2", tag="sin2")
                    low = ring2[t % 3]
                    src = init_bc[2] if t == 0 else ps_prev[2]
                    nc.vector.tensor_add(s_in[:], src[:], low[:])

                # --- err / fc staging (gpsimd; SBUF only) ---
                if err_tile[l] is None:
                    err_tile[l] = errp[l].tile([NY, EB * Bc], f32, name=f"errt{l}", tag=f"errt{l}")
                et = err_tile[l]
                eb = slice((t % EB) * Bc, (t % EB) * Bc + Bc)
                if past:
                    nc.gpsimd.tensor_sub(et[:, eb], s_in[0:NY, 0:Bc], yt[:, tb])
                else:
                    nc.gpsimd.tensor_copy(et[:, eb], s_in[0:NY, 0:Bc])

                # --- tanh (+ teacher splice) ---
                th = thp[l].tile([128, KC * Bc], bf16, name=f"th{l}", tag=f"tht{l}")
                nc.scalar.activation(th[:], s_in[:], AF.Tanh)
                if past:
                    nc.gpsimd.tensor_copy(th[0:NY, 0:Bc], ty[:, tb])

                # --- matmuls: ps[:, ncc*Bc:+Bc] (features ncc*128..) over kc ---
                ps = psp[l].tile([128, KC * Bc], f32, name=f"ps{l}", tag=f"pst{l}")
                for ncc in range(KC):
                    nr = slice(ncc * 128, (ncc + 1) * 128)
                    ob = slice(ncc * Bc, (ncc + 1) * Bc)
                    for kc in range(KC):
                        nc.tensor.matmul(
                            ps[:, ob],
                            at[l][kc][:, nr],
                            th[:, kc * Bc : (kc + 1) * Bc],
                            start=(kc == 0),
                            stop=(kc == KC - 1),
                        )

                # --- advance ---
                if l == 0:
                    nxt = ring1[(t + 1) % 3]
                    nc.vector.tensor_add(nxt[:], ps[:], w_cur[:])
                    if t + 1 < T:
                        # next step's w = (1-d)*s_corr(t+1), off the chain
                        w_cur = wp.tile([128, KC * Bc], f32, name="w", tag="w")
                        nc.gpsimd.tensor_mul(w_cur[:], efull[:], nxt[:])
                        if t + 1 < T_PAST:
                            nc.gpsimd.tensor_copy(
                                w_cur[0:NY, 0:Bc], dyt[:, (t + 1) * Bc : (t + 2) * Bc]
                            )
                else:
                    ps_prev[l] = ps

                # --- stage DMA out ---
                if (t % EB) == EB - 1 or t == T - 1:
                    nb = (t % EB) + 1
                    t0 = t - (nb - 1)
                    nc.sync.dma_start(eo_h[l, :, t0 : t0 + nb, :], et[:, 0 : nb * Bc])
                    err_tile[l] = None

    nc.compile()
    return nc


def _prep_inputs(Y, init_state, A, lstm_gate):
    d = np.clip(lstm_gate.astype(np.float32), 0.0, 1.0)
    Af = A.astype(np.float32).copy()
    Af[0] = d[:, None] * Af[0]  # gate folded into level-0 weights
    at_np = np.ascontiguousarray(Af.transpose(0, 2, 1)).astype(ml_dtypes.bfloat16)
    efull = np.empty((128, KC * Bc), np.float32)
    for kc in range(KC):
        efull[:, kc * Bc : (kc + 1) * Bc] = 1.0 - d[kc * 128 : (kc + 1) * 128, None]
    init = np.empty((128, D * KC), np.float32)
    for l in range(D):
        for kc in range(KC):
            init[:, l * KC + kc] = init_state[l, 0, kc * 128 : (kc + 1) * 128]
    one_minus_d0 = (1.0 - d[:NY]).astype(np.float32)

    in_maps = []
    for c in range(NCORES):
        Ys = Y[:, c * Bc : (c + 1) * Bc, :].astype(np.float32)  # (T_PAST, Bc, NY)
        yt = np.ascontiguousarray(Ys.transpose(2, 0, 1))  # (NY, T_PAST, Bc)
        ty = np.tanh(yt).astype(ml_dtypes.bfloat16)
        dyt = np.ascontiguousarray(yt * one_minus_d0[:, None, None])
        in_maps.append({
            "at_h": at_np,
            "ty_h": ty,
            "yt_h": yt,
            "dyt_h": dyt,
            "efull_h": efull,
            "init_h": init,
        })
    return in_maps


def kernel(Y, init_state, A, lstm_gate, forecast_horizon):
    global _nc_cache
    Y = np.asarray(Y)
    init_state = np.asarray(init_state)
    A = np.asarray(A)
    lstm_gate = np.asarray(lstm_gate)
    assert int(forecast_horizon) == T_FC, f"kernel hardcodes T_FC={T_FC}"
    assert Y.shape == (T_PAST, B, NY) and A.shape == (D, NS, NS)

    if _nc_cache is None:
        _nc_cache = build_nc()
    nc = _nc_cache

    in_maps = _prep_inputs(Y, init_state, A, lstm_gate)
    res = run_bass_kernel_spmd(nc, in_maps, list(range(NCORES)))

    eo = np.stack([res.results[c]["eo_h"] for c in range(NCORES)])  # (8, D, NY, T, Bc)
    out = eo.transpose(1, 3, 0, 4, 2).reshape(D, T, B, NY)
    return np.ascontiguousarray(out.astype(Y.dtype))
